# revision 5
# baseline (speedup 1.0000x reference)
"""DirectForce GNN message-passing kernel for 8 Trainium2 NeuronCores.

Structure (v2)
--------------
Device (8 cores, edge-sharded, weights replicated), per 512-edge row-tile,
activations feature-on-partition (transposed), all matmuls bf16:

  PE   : z1 = W1.x (16 MMs), z2 = W2'.h1' (16 MMs).  No L3 matmul.
  DVE  : h1' = POLY1(z1) -- a single 8-stage custom DVE op evaluating
         2*softplus(z) - a0 = z + p(z^2) straight out of PSUM (softplus is
         z/2 + g(z^2) with g entire, so an even degree-6 polynomial fits to
         +-0.006; the a0 constant and the factor 2 are folded into W2/b2 on
         the host).  Also u2 = exp(b2'')*E2 per chunk (per-partition scalars).
  ACT  : E2 = Exp(z2) from PSUM (two 2-bank ops), h2 = Ln(u2 + 1) in one
         wide 4-chunk op -- the exact softplus for the output layer.
  Pool : acc = sum_kc w3_kc * h2_kc (one TS + three STT ops).
  DMA  : acc tiles [128, 512] stream to DRAM; the host closes the 128-way
         partition reduction (the old ones-matmul) plus b3 folds.

Host: magnitude partner-pairing lexsort (exact reference transcription),
symmetrization, segment-sum -- O(E) index work, ~0.3% of the FLOPs.

Every TPB instruction encodes at most ONE semaphore wait; after Tile
scheduling we legalize by hoisting excess waits onto same-engine NOPs.
"""

import numpy as np

E = 262144
D = 512
N_CORES = 8
RPC = E // N_CORES          # edges per core = 32768
RT = 512                    # edges per row-tile
NT = RPC // RT              # 64 row-tiles per core
KC = D // 128               # 4 feature chunks
LOG2 = float(np.log(2.0))

POLY_B = 25.0               # fit interval for w = z^2

_CACHE = {}


def _poly_coefs():
    """Least-squares fit of 2*softplus(z) - z as cubic in w = z^2 on
    chebyshev nodes over [0, POLY_B]. Returns (a3, a2, a1, a0)."""
    if "poly" not in _CACHE:
        w = (np.cos(np.linspace(0, np.pi, 4000)) + 1) / 2 * POLY_B
        z = np.sqrt(w)
        t = 2 * np.logaddexp(0, z) - z
        V = np.vander(w, 4)
        coef, *_ = np.linalg.lstsq(V, t, rcond=None)
        _CACHE["poly"] = tuple(float(c) for c in coef)
    return _CACHE["poly"]


def _legalize_waits(nc):
    """Every TPB instruction carries at most one sync wait; hoist extras onto
    same-engine NOPs placed immediately before the offender."""
    import concourse.mybir as mybir

    eng_map = {
        mybir.EngineType.PE: nc.tensor,
        mybir.EngineType.Activation: nc.scalar,
        mybir.EngineType.DVE: nc.vector,
        mybir.EngineType.Pool: nc.gpsimd,
        mybir.EngineType.SP: nc.sync,
    }
    n_nops = 0
    for blk in nc.main_func.blocks:
        offenders = [
            ins for ins in blk.instructions
            if ins.sync_info is not None and len(ins.sync_info.on_wait) > 1
        ]
        for ins in offenders:
            si = ins.sync_info
            waits = list(si.on_wait)
            si.on_wait = [waits[-1]]
            eng = eng_map.get(ins.engine, nc.sync)
            idx = blk.instructions.index(ins)
            for w in waits[:-1]:
                nop_ins = eng.nop(nofuse=True).ins
                nop_ins.sync_info = mybir.SyncInfo(on_wait=[w], on_update=[])
                cur = nc.cur_bb.bb
                cur.instructions.remove(nop_ins)
                blk.instructions.insert(idx, nop_ins)
                idx += 1
                n_nops += 1
    return n_nops


def _build_program():
    import concourse.bass as bass
    import concourse.mybir as mybir
    import concourse.tile as tile

    a3, a2, a1, a0 = _poly_coefs()

    bf = mybir.dt.bfloat16
    f32 = mybir.dt.float32
    AF = mybir.ActivationFunctionType
    ALU = mybir.AluOpType

    nc = bass.Bass()
    xt = nc.dram_tensor("xt", [D, RPC], bf, kind="ExternalInput")
    # packed weights: chunk c holds rows d = c*128 + p; cols 0:512 = W1,
    # 512:1024 = W2' ( = W2/2 )
    wp = nc.dram_tensor("wp", [128, KC, 2 * D], bf, kind="ExternalInput")
    # bpack cols: 0 = g = exp(b2''), 1 = w3, 2 = 1.0 (Ln bias), 3 = b1
    bp = nc.dram_tensor("bp", [128, KC, 4], f32, kind="ExternalInput")
    magp = nc.dram_tensor("magp", [NT, 128, RT], f32, kind="ExternalOutput")

    xt_v = xt.rearrange("(c p) r -> p c r", p=128)  # [128, KC, RPC]

    with tile.TileContext(nc) as tc:
        with (
            tc.tile_pool(name="singles", bufs=1) as singles,
            tc.tile_pool(name="xp", bufs=3) as xp,
            tc.tile_pool(name="wsq", bufs=2) as wsqp,
            tc.tile_pool(name="t1p", bufs=2) as t1p,
            tc.tile_pool(name="t2p", bufs=2) as t2p,
            tc.tile_pool(name="t3p", bufs=2) as t3p,
            tc.tile_pool(name="h1p", bufs=2) as h1p,
            tc.tile_pool(name="e2p", bufs=2) as e2p,
            tc.tile_pool(name="u2p", bufs=2) as u2p,
            tc.tile_pool(name="h2p", bufs=2) as h2p,
            tc.tile_pool(name="tw", bufs=2) as twp,
            tc.tile_pool(name="accp", bufs=2) as accp,
            tc.tile_pool(name="ps1p", bufs=2, space="PSUM") as ps1p,
            tc.tile_pool(name="ps2p", bufs=2, space="PSUM") as ps2p,
        ):
            wpack = singles.tile([128, KC, 2 * D], bf)
            nc.sync.dma_start(out=wpack, in_=wp[:, :, :])
            bpack = singles.tile([128, KC, 4], f32)
            nc.sync.dma_start(out=bpack, in_=bp[:, :, :])

            h1_tiles = {}

            def emit_l1(rt):
                # layer 1: z1 = W1.x (PSUM pair-tiles); then
                # h1' = z1 + ((a3*w + a2)*w + a1)*w, w = z1^2:
                #   ACT: w = Square(z1)   DVE: t1 = a3*w + a2 (dual TS)
                #   Pool: t2 = t1*w ; t2 += a1 ; t3 = t2*w
                #   DVE: h1 = t3 + z1 (TT, psum read, bf16 out)
                x_all = xp.tile([128, KC, RT], bf, tag="x")
                nc.sync.dma_start(out=x_all,
                                  in_=xt_v[:, :, rt * RT:(rt + 1) * RT])
                h1 = h1p.tile([128, KC, RT], bf, tag="h1")
                h1_tiles[rt] = h1
                for jp in range(KC // 2):
                    ps1 = ps1p.tile([128, 2, RT], f32, tag="ps1")
                    for sub in range(2):
                        jc = 2 * jp + sub
                        for dc in range(KC):
                            nc.tensor.matmul(
                                ps1[:, sub, :],
                                wpack[:, dc, jc * 128:(jc + 1) * 128],
                                x_all[:, dc, :],
                                start=(dc == 0), stop=(dc == KC - 1),
                            )
                    wsq = wsqp.tile([128, 2, RT], f32, tag="wsq")
                    nc.scalar.activation(wsq, ps1, AF.Square)
                    t1 = t1p.tile([128, 2, RT], f32, tag="t1")
                    nc.vector.tensor_scalar(t1, wsq, a3, a2, ALU.mult,
                                            ALU.add)
                    t2 = t2p.tile([128, 2, RT], f32, tag="t2")
                    nc.gpsimd.tensor_tensor(t2, t1, wsq, ALU.mult)
                    nc.gpsimd.tensor_scalar(t2, t2, a1, None, ALU.add)
                    t3 = t3p.tile([128, 2, RT], f32, tag="t3")
                    nc.gpsimd.tensor_tensor(t3, t2, wsq, ALU.mult)
                    nc.vector.tensor_tensor(h1[:, 2 * jp:2 * jp + 2, :], t3,
                                            ps1, ALU.add)

            def emit_l2(rt):
                # layer 2: z2 = W2'.h1' (PSUM); E2 = Exp(z2) on ACT
                h1 = h1_tiles.pop(rt)
                e2 = e2p.tile([128, KC, RT], f32, tag="e2")
                for kp in range(KC // 2):
                    ps2 = ps2p.tile([128, 2, RT], f32, tag="ps2")
                    for sub in range(2):
                        kc = 2 * kp + sub
                        for jc in range(KC):
                            nc.tensor.matmul(
                                ps2[:, sub, :],
                                wpack[:, jc, D + kc * 128:D + (kc + 1) * 128],
                                h1[:, jc, :],
                                start=(jc == 0), stop=(jc == KC - 1),
                            )
                    nc.scalar.activation(e2[:, 2 * kp:2 * kp + 2, :], ps2,
                                         AF.Exp)

                # u2 = g * E2 per chunk (DVE), h2 = Ln(u2 + 1) wide (ACT)
                u2 = u2p.tile([128, KC, RT], f32, tag="u2")
                for kc in range(KC):
                    nc.vector.tensor_scalar(u2[:, kc, :], e2[:, kc, :],
                                            bpack[:, kc, 0:1], None, ALU.mult)
                h2 = h2p.tile([128, KC, RT], bf, tag="h2")
                nc.scalar.activation(h2, u2, AF.Ln, bias=bpack[:, 0, 2:3])

                # acc = sum_kc w3_kc * h2_kc (DVE bf16 + Pool adds)
                tw = twp.tile([128, KC, RT], bf, tag="tw")
                for kc in range(KC):
                    nc.vector.tensor_scalar(tw[:, kc, :], h2[:, kc, :],
                                            bpack[:, kc, 1:2], None, ALU.mult)
                acc = accp.tile([128, RT], f32, tag="acc")
                nc.gpsimd.tensor_tensor(acc, tw[:, 0, :], tw[:, 1, :],
                                        ALU.add)
                tb = accp.tile([128, RT], f32, tag="tb")
                nc.gpsimd.tensor_tensor(tb, tw[:, 2, :], tw[:, 3, :],
                                        ALU.add)
                nc.vector.tensor_tensor(acc, acc, tb, ALU.add)
                nc.sync.dma_start(out=magp[rt, :, :], in_=acc)

            for rt in range(NT + 1):
                if rt < NT:
                    emit_l1(rt)
                if rt >= 1:
                    emit_l2(rt - 1)

    _legalize_waits(nc)
    return nc


def _get_program():
    if "prog" not in _CACHE:
        _CACHE["prog"] = _build_program()
    return _CACHE["prog"]


def _run_mlp(edge_emb, W1, b1, W2, b2, W3, b3, trace=False):
    """Run the edge MLP on 8 NeuronCores; returns mag [E] fp32."""
    import ml_dtypes
    from concourse.bass_utils import run_bass_kernel_spmd

    bf = ml_dtypes.bfloat16

    W1 = np.asarray(W1, np.float32)
    W2 = np.asarray(W2, np.float32)
    W3 = np.asarray(W3, np.float32)
    b1 = np.asarray(b1, np.float32)
    b2 = np.asarray(b2, np.float32)
    b3 = np.asarray(b3, np.float32)

    nc = _get_program()
    a3, a2, a1, a0 = _poly_coefs()

    # host-side constant folds (see module docstring)
    colsum_w2 = W2.sum(axis=0)
    b2pp = b2 + (0.5 * a0 - LOG2) * colsum_w2     # z2 true-bias shift
    b3p = float(b3[0] - LOG2 * W3.sum(axis=0)[0])  # -ln2 of h2 fold

    wpack = np.empty((128, KC, 2 * D), np.float32)
    for c in range(KC):
        rows = slice(c * 128, (c + 1) * 128)
        wpack[:, c, 0:D] = W1[rows, :]
        wpack[:, c, D:2 * D] = 0.5 * W2[rows, :]
    wpack = np.ascontiguousarray(wpack.astype(bf))

    bpack = np.empty((128, KC, 4), np.float32)
    for c in range(KC):
        rows = slice(c * 128, (c + 1) * 128)
        bpack[:, c, 0] = np.exp(b2pp[rows]).astype(np.float32)
        bpack[:, c, 1] = W3[rows, 0]
        bpack[:, c, 2] = 1.0
        bpack[:, c, 3] = b1[rows]
    bpack = np.ascontiguousarray(bpack)

    emb = np.asarray(edge_emb, np.float32)
    in_maps = []
    for c in range(N_CORES):
        shard = emb[c * RPC:(c + 1) * RPC, :]
        xt_shard = np.ascontiguousarray(shard.T.astype(bf, copy=False))
        in_maps.append({"xt": xt_shard, "wp": wpack, "bp": bpack})

    kwargs = {}
    if trace:
        _register_ntff_hook()
        kwargs["trace"] = True
    res = run_bass_kernel_spmd(nc, in_maps, core_ids=list(range(N_CORES)),
                               **kwargs)
    shards = []
    for c in range(N_CORES):
        mp = res.results[c]["magp"]          # [NT, 128, RT]
        shards.append(mp.sum(axis=1, dtype=np.float64).reshape(-1))
    mag_out = np.concatenate(shards).astype(np.float32)
    if trace:
        print(f"HW exec time: {res.exec_time_ns} ns "
              f"(mean {res.mean_exec_time_ns} ns across cores)")
    return mag_out + np.float32(b3p)


def _register_ntff_hook():
    """The image's antenv lacks axon_hooks; synthesize it so trace=True can
    capture NTFF profiles through the axon PJRT library."""
    import sys, types
    if "antenv.axon_hooks" in sys.modules:
        return
    mod = types.ModuleType("antenv.axon_hooks")
    state = {"hook": None}
    mod.set_axon_ntff_profile_hook = lambda h: state.__setitem__("hook", h)
    mod.get_axon_ntff_profile_hook = lambda: state["hook"]
    sys.modules["antenv.axon_hooks"] = mod
    import antenv
    antenv.axon_hooks = mod
    try:
        from trn_agent_boot.trn_boot import _ntff_profile_via_ctypes
        mod.set_axon_ntff_profile_hook(
            _ntff_profile_via_ctypes("/opt/axon/libaxon_pjrt.so"))
    except Exception:
        pass


def _forces_from_mag(mag, edge_vectors, edge_lengths, edge_index,
                     edge_cell_shift, N):
    """Exact numpy transcription of the reference pairing + segment sum."""
    uv = np.asarray(edge_vectors, np.float32) / np.asarray(
        edge_lengths, np.float32)[:, None]
    s = np.asarray(edge_cell_shift, np.int64)
    s0, s1, s2 = s[:, 0], s[:, 1], s[:, 2]
    c = np.asarray(edge_index[0], np.int64)
    n = np.asarray(edge_index[1], np.int64)
    fwd = c * N + n
    rev = n * N + c
    N2 = N * N
    conds = [
        (s0 == 0) & (s1 == 0) & (s2 == 0),
        (s0 == -1) & (s1 == 0) & (s2 == 0),
        (s1 == -1) & (s2 == 0),
        (s2 == -1),
        (s0 == 1) & (s1 == 0) & (s2 == 0),
        (s1 == 1) & (s2 == 0),
        (s2 == 1),
    ]
    keys = [
        fwd,
        fwd,
        (s0 + 2) * N2 + fwd,
        (s0 + 6) * (s1 + 2) * N2 + fwd,
        rev,
        (-s0 + 2) * N2 + rev,
        (-s0 + 6) * (-s1 + 2) * N2 + rev,
    ]
    cat = np.select(conds, [np.full_like(c, i) for i in range(7)],
                    np.full_like(c, 6))
    key = np.select(conds, keys, rev)
    perm = np.lexsort((key, cat))
    mag_s = mag[perm]
    uv_s = uv[perm]
    c_s = c[perm]
    n_s = n[perm]
    cat_s = cat[perm]
    perm2 = np.lexsort((n_s * N + c_s, cat_s))
    M = int(np.sum((cat_s >= 1) & (cat_s <= 3)))
    idx = np.arange(E, dtype=np.int64)
    partner = np.where(cat_s == 0, perm2,
                       np.where(cat_s <= 3, idx + M, idx - M))
    mag_f = (mag_s + mag_s[partner]) * np.float32(0.5)
    contrib = mag_f[:, None] * uv_s
    forces = np.empty((N, 3), np.float32)
    for d in range(3):
        forces[:, d] = np.bincount(c_s, weights=contrib[:, d],
                                   minlength=N).astype(np.float32)
    return forces


def kernel(edge_emb, edge_vectors, edge_lengths, W1, b1, W2, b2, W3, b3,
           edge_index, edge_cell_shift, atom_count, _trace=False):
    N = int(atom_count)
    mag = _run_mlp(edge_emb, W1, b1, W2, b2, W3, b3, trace=_trace)
    return _forces_from_mag(mag, edge_vectors, edge_lengths, edge_index,
                            edge_cell_shift, N)


# revision 7
# speedup vs baseline: 3.7238x; 3.7238x over previous
"""DirectForce GNN message-passing kernel for 8 Trainium2 NeuronCores.

Structure (v2)
--------------
Device (8 cores, edge-sharded, weights replicated), per 512-edge row-tile,
activations feature-on-partition (transposed), all matmuls bf16:

  PE   : z1 = W1.x (16 MMs), z2 = W2'.h1' (16 MMs).  No L3 matmul.
  DVE  : h1' = POLY1(z1) -- a single 8-stage custom DVE op evaluating
         2*softplus(z) - a0 = z + p(z^2) straight out of PSUM (softplus is
         z/2 + g(z^2) with g entire, so an even degree-6 polynomial fits to
         +-0.006; the a0 constant and the factor 2 are folded into W2/b2 on
         the host).  Also u2 = exp(b2'')*E2 per chunk (per-partition scalars).
  ACT  : E2 = Exp(z2) from PSUM (two 2-bank ops), h2 = Ln(u2 + 1) in one
         wide 4-chunk op -- the exact softplus for the output layer.
  Pool : acc = sum_kc w3_kc * h2_kc (one TS + three STT ops).
  DMA  : acc tiles [128, 512] stream to DRAM; the host closes the 128-way
         partition reduction (the old ones-matmul) plus b3 folds.

Host: magnitude partner-pairing lexsort (exact reference transcription),
symmetrization, segment-sum -- O(E) index work, ~0.3% of the FLOPs.

Every TPB instruction encodes at most ONE semaphore wait; after Tile
scheduling we legalize by hoisting excess waits onto same-engine NOPs.
"""

import numpy as np

E = 262144
D = 512
N_CORES = 8
RPC = E // N_CORES          # edges per core = 32768
RT = 512                    # edges per row-tile
NT = RPC // RT              # 64 row-tiles per core
KC = D // 128               # 4 feature chunks
LOG2 = float(np.log(2.0))

POLY_B = 25.0               # fit interval for w = z^2

_CACHE = {}


def _poly_coefs():
    """Least-squares fit of 2*softplus(z) - z as cubic in w = z^2 on
    chebyshev nodes over [0, POLY_B]. Returns (a3, a2, a1, a0)."""
    if "poly" not in _CACHE:
        w = (np.cos(np.linspace(0, np.pi, 4000)) + 1) / 2 * POLY_B
        z = np.sqrt(w)
        t = 2 * np.logaddexp(0, z) - z
        V = np.vander(w, 4)
        coef, *_ = np.linalg.lstsq(V, t, rcond=None)
        _CACHE["poly"] = tuple(float(c) for c in coef)
    return _CACHE["poly"]


def _legalize_waits(nc):
    """Every TPB instruction carries at most one sync wait; hoist extras onto
    same-engine NOPs placed immediately before the offender."""
    import concourse.mybir as mybir

    eng_map = {
        mybir.EngineType.PE: nc.tensor,
        mybir.EngineType.Activation: nc.scalar,
        mybir.EngineType.DVE: nc.vector,
        mybir.EngineType.Pool: nc.gpsimd,
        mybir.EngineType.SP: nc.sync,
    }
    n_nops = 0
    for blk in nc.main_func.blocks:
        offenders = [
            ins for ins in blk.instructions
            if ins.sync_info is not None and len(ins.sync_info.on_wait) > 1
        ]
        for ins in offenders:
            si = ins.sync_info
            waits = list(si.on_wait)
            si.on_wait = [waits[-1]]
            eng = eng_map.get(ins.engine, nc.sync)
            idx = blk.instructions.index(ins)
            for w in waits[:-1]:
                nop_ins = eng.nop(nofuse=True).ins
                nop_ins.sync_info = mybir.SyncInfo(on_wait=[w], on_update=[])
                cur = nc.cur_bb.bb
                cur.instructions.remove(nop_ins)
                blk.instructions.insert(idx, nop_ins)
                idx += 1
                n_nops += 1
    return n_nops


def _build_program():
    import concourse.bass as bass
    import concourse.mybir as mybir
    import concourse.tile as tile

    a3, a2, a1, a0 = _poly_coefs()

    bf = mybir.dt.bfloat16
    f32 = mybir.dt.float32
    AF = mybir.ActivationFunctionType
    ALU = mybir.AluOpType

    nc = bass.Bass()
    xt = nc.dram_tensor("xt", [D, RPC], bf, kind="ExternalInput")
    # packed weights: chunk c holds rows d = c*128 + p; cols 0:512 = W1,
    # 512:1024 = W2' ( = W2/2 )
    wp = nc.dram_tensor("wp", [128, KC, 2 * D], bf, kind="ExternalInput")
    # bpack cols: 0 = g = exp(b2''), 1 = w3, 2 = 1.0 (Ln bias), 3 = b1
    bp = nc.dram_tensor("bp", [128, KC, 4], f32, kind="ExternalInput")
    magp = nc.dram_tensor("magp", [NT, 128, RT], f32, kind="ExternalOutput")

    xt_v = xt.rearrange("(c p) r -> p c r", p=128)  # [128, KC, RPC]

    with tile.TileContext(nc) as tc:
        with (
            tc.tile_pool(name="singles", bufs=1) as singles,
            tc.tile_pool(name="xp", bufs=3) as xp,
            tc.tile_pool(name="wsq", bufs=2) as wsqp,
            tc.tile_pool(name="t1p", bufs=2) as t1p,
            tc.tile_pool(name="zbp", bufs=2) as zbp,
            tc.tile_pool(name="w2p", bufs=2) as w2pp,
            tc.tile_pool(name="cp", bufs=2) as cpp,
            tc.tile_pool(name="h1p", bufs=2) as h1p,
            tc.tile_pool(name="e2p", bufs=2) as e2p,
            tc.tile_pool(name="h2p", bufs=2) as h2p,
            tc.tile_pool(name="sacc", bufs=8) as saccp,
            tc.tile_pool(name="ps1p", bufs=1, space="PSUM") as ps1p,
            tc.tile_pool(name="ps2p", bufs=1, space="PSUM") as ps2p,
        ):
            wpack = singles.tile([128, KC, 2 * D], bf)
            nc.sync.dma_start(out=wpack, in_=wp[:, :, :])
            bpack = singles.tile([128, KC, 4], f32)
            nc.sync.dma_start(out=bpack, in_=bp[:, :, :])

            h1_tiles = {}

            def emit_l1(rt):
                # layer 1: z1 = W1.x (one 4-bank PSUM tile); then
                # h1' = z1 + a1*w + (a3*w + a2)*w^2, w = z1^2:
                #   ACT : w  = Square(z1)            (wide, bf16 out)
                #   DVE : t1 = a3*w + a2             (dual TS)
                #   DVE : zb = a1*w + z1             (STT, psum read)
                #   DVE : w2 = w*w                   (TT)
                #   DVE : c  = t1*w2                 (TT)
                #   Pool: h1 = c + zb                (TT, bf16)
                x_all = xp.tile([128, KC, RT], bf, tag="x")
                nc.sync.dma_start(out=x_all,
                                  in_=xt_v[:, :, rt * RT:(rt + 1) * RT])
                h1 = h1p.tile([128, KC, RT], bf, tag="h1")
                h1_tiles[rt] = h1
                ps1 = ps1p.tile([128, KC, RT], f32, tag="ps1")
                for jc in range(KC):
                    for dc in range(KC):
                        nc.tensor.matmul(
                            ps1[:, jc, :],
                            wpack[:, dc, jc * 128:(jc + 1) * 128],
                            x_all[:, dc, :],
                            start=(dc == 0), stop=(dc == KC - 1),
                        )
                wsq = wsqp.tile([128, KC, RT], bf, tag="wsq")
                nc.scalar.activation(wsq, ps1, AF.Square)
                t1 = t1p.tile([128, KC, RT], bf, tag="t1")
                nc.vector.tensor_scalar(t1, wsq, a3, a2, ALU.mult, ALU.add)
                zb = zbp.tile([128, KC, RT], bf, tag="zb")
                nc.vector.scalar_tensor_tensor(zb, in0=wsq, scalar=a1,
                                               in1=ps1, op0=ALU.mult,
                                               op1=ALU.add)
                w2 = w2pp.tile([128, KC, RT], bf, tag="w2")
                nc.vector.tensor_tensor(w2, wsq, wsq, ALU.mult)
                c = cpp.tile([128, KC, RT], bf, tag="c")
                nc.vector.tensor_tensor(c, t1, w2, ALU.mult)
                nc.gpsimd.tensor_tensor(h1, c, zb, ALU.add)

            def emit_l2(rt):
                # layer 2: z2 = W2'.h1'; E2 = Exp(z2 + b2'') per chunk (ACT);
                # h2 = Ln(E2 + 1) wide (ACT); acc = sum_kc w3_kc*h2_kc (DVE)
                h1 = h1_tiles.pop(rt)
                ps2 = ps2p.tile([128, KC, RT], f32, tag="ps2")
                for kc in range(KC):
                    for jc in range(KC):
                        nc.tensor.matmul(
                            ps2[:, kc, :],
                            wpack[:, jc, D + kc * 128:D + (kc + 1) * 128],
                            h1[:, jc, :],
                            start=(jc == 0), stop=(jc == KC - 1),
                        )
                e2 = e2p.tile([128, KC, RT], bf, tag="e2")
                for kc in range(KC):
                    nc.scalar.activation(e2[:, kc, :], ps2[:, kc, :], AF.Exp,
                                         bias=bpack[:, kc, 0:1])
                h2 = h2p.tile([128, KC, RT], bf, tag="h2")
                nc.scalar.activation(h2, e2, AF.Ln, bias=bpack[:, 0, 2:3])

                s1 = saccp.tile([128, RT], bf, tag="s1")
                nc.vector.tensor_scalar(s1, h2[:, 0, :], bpack[:, 0, 1:2],
                                        None, ALU.mult)
                s2 = saccp.tile([128, RT], bf, tag="s2")
                nc.vector.scalar_tensor_tensor(s2, in0=h2[:, 1, :],
                                               scalar=bpack[:, 1, 1:2],
                                               in1=s1, op0=ALU.mult,
                                               op1=ALU.add)
                s3 = saccp.tile([128, RT], bf, tag="s3")
                nc.vector.scalar_tensor_tensor(s3, in0=h2[:, 2, :],
                                               scalar=bpack[:, 2, 1:2],
                                               in1=s2, op0=ALU.mult,
                                               op1=ALU.add)
                acc = saccp.tile([128, RT], f32, tag="acc")
                nc.vector.scalar_tensor_tensor(acc, in0=h2[:, 3, :],
                                               scalar=bpack[:, 3, 1:2],
                                               in1=s3, op0=ALU.mult,
                                               op1=ALU.add)
                nc.sync.dma_start(out=magp[rt, :, :], in_=acc)

            for rt in range(NT + 1):
                if rt < NT:
                    emit_l1(rt)
                if rt >= 1:
                    emit_l2(rt - 1)

    _legalize_waits(nc)
    return nc


def _get_program():
    if "prog" not in _CACHE:
        _CACHE["prog"] = _build_program()
    return _CACHE["prog"]


def _run_mlp(edge_emb, W1, b1, W2, b2, W3, b3, trace=False):
    """Run the edge MLP on 8 NeuronCores; returns mag [E] fp32."""
    import ml_dtypes
    from concourse.bass_utils import run_bass_kernel_spmd

    bf = ml_dtypes.bfloat16

    W1 = np.asarray(W1, np.float32)
    W2 = np.asarray(W2, np.float32)
    W3 = np.asarray(W3, np.float32)
    b1 = np.asarray(b1, np.float32)
    b2 = np.asarray(b2, np.float32)
    b3 = np.asarray(b3, np.float32)

    nc = _get_program()
    a3, a2, a1, a0 = _poly_coefs()

    # host-side constant folds (see module docstring)
    colsum_w2 = W2.sum(axis=0)
    b2pp = b2 + (0.5 * a0 - LOG2) * colsum_w2     # z2 true-bias shift
    b3p = float(b3[0] - LOG2 * W3.sum(axis=0)[0])  # -ln2 of h2 fold

    wpack = np.empty((128, KC, 2 * D), np.float32)
    for c in range(KC):
        rows = slice(c * 128, (c + 1) * 128)
        wpack[:, c, 0:D] = W1[rows, :]
        wpack[:, c, D:2 * D] = 0.5 * W2[rows, :]
    wpack = np.ascontiguousarray(wpack.astype(bf))

    bpack = np.empty((128, KC, 4), np.float32)
    for c in range(KC):
        rows = slice(c * 128, (c + 1) * 128)
        bpack[:, c, 0] = b2pp[rows]
        bpack[:, c, 1] = W3[rows, 0]
        bpack[:, c, 2] = 1.0
        bpack[:, c, 3] = b1[rows]
    bpack = np.ascontiguousarray(bpack)

    emb = np.asarray(edge_emb, np.float32)
    in_maps = []
    for c in range(N_CORES):
        shard = emb[c * RPC:(c + 1) * RPC, :]
        xt_shard = np.ascontiguousarray(shard.T.astype(bf, copy=False))
        in_maps.append({"xt": xt_shard, "wp": wpack, "bp": bpack})

    kwargs = {}
    if trace:
        _register_ntff_hook()
        kwargs["trace"] = True
    res = run_bass_kernel_spmd(nc, in_maps, core_ids=list(range(N_CORES)),
                               **kwargs)
    shards = []
    for c in range(N_CORES):
        mp = res.results[c]["magp"]          # [NT, 128, RT]
        shards.append(mp.sum(axis=1, dtype=np.float64).reshape(-1))
    mag_out = np.concatenate(shards).astype(np.float32)
    if trace:
        print(f"HW exec time: {res.exec_time_ns} ns "
              f"(mean {res.mean_exec_time_ns} ns across cores)")
    return mag_out + np.float32(b3p)


def _register_ntff_hook():
    """The image's antenv lacks axon_hooks; synthesize it so trace=True can
    capture NTFF profiles through the axon PJRT library."""
    import sys, types
    if "antenv.axon_hooks" in sys.modules:
        return
    mod = types.ModuleType("antenv.axon_hooks")
    state = {"hook": None}
    mod.set_axon_ntff_profile_hook = lambda h: state.__setitem__("hook", h)
    mod.get_axon_ntff_profile_hook = lambda: state["hook"]
    sys.modules["antenv.axon_hooks"] = mod
    import antenv
    antenv.axon_hooks = mod
    try:
        from trn_agent_boot.trn_boot import _ntff_profile_via_ctypes
        mod.set_axon_ntff_profile_hook(
            _ntff_profile_via_ctypes("/opt/axon/libaxon_pjrt.so"))
    except Exception:
        pass


def _forces_from_mag(mag, edge_vectors, edge_lengths, edge_index,
                     edge_cell_shift, N):
    """Exact numpy transcription of the reference pairing + segment sum."""
    uv = np.asarray(edge_vectors, np.float32) / np.asarray(
        edge_lengths, np.float32)[:, None]
    s = np.asarray(edge_cell_shift, np.int64)
    s0, s1, s2 = s[:, 0], s[:, 1], s[:, 2]
    c = np.asarray(edge_index[0], np.int64)
    n = np.asarray(edge_index[1], np.int64)
    fwd = c * N + n
    rev = n * N + c
    N2 = N * N
    conds = [
        (s0 == 0) & (s1 == 0) & (s2 == 0),
        (s0 == -1) & (s1 == 0) & (s2 == 0),
        (s1 == -1) & (s2 == 0),
        (s2 == -1),
        (s0 == 1) & (s1 == 0) & (s2 == 0),
        (s1 == 1) & (s2 == 0),
        (s2 == 1),
    ]
    keys = [
        fwd,
        fwd,
        (s0 + 2) * N2 + fwd,
        (s0 + 6) * (s1 + 2) * N2 + fwd,
        rev,
        (-s0 + 2) * N2 + rev,
        (-s0 + 6) * (-s1 + 2) * N2 + rev,
    ]
    cat = np.select(conds, [np.full_like(c, i) for i in range(7)],
                    np.full_like(c, 6))
    key = np.select(conds, keys, rev)
    perm = np.lexsort((key, cat))
    mag_s = mag[perm]
    uv_s = uv[perm]
    c_s = c[perm]
    n_s = n[perm]
    cat_s = cat[perm]
    perm2 = np.lexsort((n_s * N + c_s, cat_s))
    M = int(np.sum((cat_s >= 1) & (cat_s <= 3)))
    idx = np.arange(E, dtype=np.int64)
    partner = np.where(cat_s == 0, perm2,
                       np.where(cat_s <= 3, idx + M, idx - M))
    mag_f = (mag_s + mag_s[partner]) * np.float32(0.5)
    contrib = mag_f[:, None] * uv_s
    forces = np.empty((N, 3), np.float32)
    for d in range(3):
        forces[:, d] = np.bincount(c_s, weights=contrib[:, d],
                                   minlength=N).astype(np.float32)
    return forces


def kernel(edge_emb, edge_vectors, edge_lengths, W1, b1, W2, b2, W3, b3,
           edge_index, edge_cell_shift, atom_count, _trace=False):
    N = int(atom_count)
    mag = _run_mlp(edge_emb, W1, b1, W2, b2, W3, b3, trace=_trace)
    return _forces_from_mag(mag, edge_vectors, edge_lengths, edge_index,
                            edge_cell_shift, N)
